# revision 10
# baseline (speedup 1.0000x reference)
"""Bounding-box discipline penalty kernel for Trainium2 (8 NeuronCores).

Reference computation:
    pred_mask = max_c(prediction_probs) > 0.3   [B, H, W]
    true_mask = max_c(expected_onehot)  > 0.5   [B, H, W]
    per-sample bboxes from the masks -> area/center penalties -> scalar mean.

Strategy (pure data parallel, B=16 over 8 cores => 2 samples/core):
  * Device: stream both tensors' shards through SBUF and compute the
    per-pixel channel max, laid out as pixmax[partition=128, 512] per
    (tensor, sample). That is the entire memory-bound part (reads 128 MiB
    per core at HBM line rate; DVE reduce overlaps the DMA).
  * Host: fold the tiny [4, 128, 512] per-core results into per-sample
    row/col maxima (exact max operations, order-independent), then do the
    O(B) bbox + penalty math exactly as the reference does.

Self-contained: hardcodes shapes from the problem spec.
"""

import numpy as np

THRESHOLD = 0.3
PENALTY_WEIGHT = 0.05

B, H, W, C = 16, 256, 256, 128
N_CORES = 8
SPC = B // N_CORES            # samples per core = 2
NST = 2 * SPC                 # sample-tensor streams per core = 4
PIX = H * W                   # 65536 pixels per sample
NPART = 128
PPP = PIX // NPART            # 512 pixels per partition
EPP = PPP * C                 # 65536 f32 elems per partition per sample
NT = 8                        # tiles per sample-tensor
F = EPP // NT                 # 8192 elems/partition per DMA (4 MiB tiles)
PXT = F // C                  # 64 pixels per partition per tile
NB = 5                        # SBUF load-buffer ring depth

_cache = {}


def _chunk_schedule():
    """Per-sample-tensor chunk sizes (f32 elems per partition).

    Uniform 8 MiB chunks, except the last sample-tensor tapers off so the
    final DVE reduce (which is serial after the last DMA lands) is short.
    """
    per_st = []
    for st in range(NST):
        if st < NST - 1:
            per_st.append([F] * NT)
        else:
            per_st.append(
                [F] * (NT - 1) + [F // 2, F // 4, F // 8, F // 16, F // 16]
            )
    for sizes in per_st:
        assert sum(sizes) == EPP
    # flat list of (st, elem offset, size, chunk-idx-in-st, is-last-of-st)
    loads = []
    for st, sizes in enumerate(per_st):
        off = 0
        for j, sz in enumerate(sizes):
            loads.append((st, off, sz, j == len(sizes) - 1))
            off += sz
    return per_st, loads


def _build_nc():
    import concourse.bass as bass
    import concourse.mybir as mybir

    f32 = mybir.dt.float32
    nc = bass.Bass()
    pred = nc.dram_tensor("pred", [SPC, NPART, EPP], f32, kind="ExternalInput")
    tru = nc.dram_tensor("tru", [SPC, NPART, EPP], f32, kind="ExternalInput")
    # pixmax per sample-tensor: [st, partition, pixel-in-partition]
    outp = nc.dram_tensor("outp", [NST, NPART, PPP], f32, kind="ExternalOutput")

    srcs = [(pred, 0), (pred, 1), (tru, 0), (tru, 1)]
    per_st, loads = _chunk_schedule()
    nloads = len(loads)
    # reduces completed (== bufree count) after finishing each st
    done_after_st = []
    acc = 0
    for sizes in per_st:
        acc += len(sizes)
        done_after_st.append(acc)

    from contextlib import ExitStack

    with ExitStack() as ctx:
        buf = [
            ctx.enter_context(nc.sbuf_tensor(f"buf{i}", [NPART, F], f32))
            for i in range(NB)
        ]
        pm = [
            ctx.enter_context(nc.sbuf_tensor(f"pm{i}", [NPART, PPP], f32))
            for i in range(2)
        ]
        lsems = [
            ctx.enter_context(nc.semaphore(f"ls{i}")) for i in range(NB)
        ]
        bufree = ctx.enter_context(nc.semaphore("bufree"))
        osems = [
            ctx.enter_context(nc.semaphore(f"os{i}")) for i in range(2)
        ]
        block = ctx.enter_context(nc.Block())
        # out-DMA count (x16) per parity, accumulated in program order
        out_counts = [0, 0]

        @block.sync
        def _(sync):
            for k, (st, off, sz, _last) in enumerate(loads):
                src, s = srcs[st]
                if k >= NB:
                    # buffer k%NB reused from load k-NB: wait for its reduce
                    sync.wait_ge(bufree, k - NB + 1)
                sync.dma_start(
                    out=buf[k % NB][:, :sz], in_=src[s, :, off : off + sz]
                ).then_inc(lsems[k % NB], 16)

        @block.vector
        def _(vector):
            for k, (st, off, sz, _last) in enumerate(loads):
                if off == 0 and st >= 2:
                    # WAR: pm[st%2] may still be DMA-ing out for st-2
                    vector.wait_ge(osems[st % 2], 16 * (st // 2))
                vector.wait_ge(lsems[k % NB], 16 * (k // NB + 1))
                vector.reduce_max(
                    out=pm[st % 2][:, off // C : (off + sz) // C],
                    in_=buf[k % NB][:, :sz].rearrange("p (a c) -> p a c", c=C),
                    axis=mybir.AxisListType.X,
                ).then_inc(bufree, 1)

        @block.scalar
        def _(scalar):
            for st in range(NST):
                sizes = per_st[st]
                par = st % 2
                if len(sizes) == NT:
                    scalar.wait_ge(bufree, done_after_st[st])
                    scalar.dma_start(out=outp[st], in_=pm[par][:]).then_inc(
                        osems[par], 16
                    )
                    out_counts[par] += 1
                else:
                    # tapered st: flush the pixels covered by the first
                    # NT-1 chunks early, then the small remainder at the end
                    head_px = sum(sizes[: NT - 1]) // C
                    base = done_after_st[st - 1] if st else 0
                    scalar.wait_ge(bufree, base + NT - 1)
                    scalar.dma_start(
                        out=outp[st, :, :head_px], in_=pm[par][:, :head_px]
                    ).then_inc(osems[par], 16)
                    out_counts[par] += 1
                    scalar.wait_ge(bufree, done_after_st[st])
                    scalar.dma_start(
                        out=outp[st, :, head_px:], in_=pm[par][:, head_px:]
                    ).then_inc(osems[par], 16)
                    out_counts[par] += 1
            scalar.wait_ge(osems[0], 16 * out_counts[0])
            scalar.wait_ge(osems[1], 16 * out_counts[1])

    return nc


def _run_device(pred_np, true_np, trace=False):
    from concourse.bass_utils import run_bass_kernel_spmd

    if "nc" not in _cache:
        _cache["nc"] = _build_nc()
    nc = _cache["nc"]

    # [B, H, W, C] -> per-core shards [SPC, 128, EPP]
    pred_sh = pred_np.reshape(N_CORES, SPC, NPART, EPP)
    true_sh = true_np.reshape(N_CORES, SPC, NPART, EPP)
    in_maps = [
        {"pred": pred_sh[i], "tru": true_sh[i]} for i in range(N_CORES)
    ]
    res = run_bass_kernel_spmd(
        nc, in_maps, core_ids=list(range(N_CORES)), trace=trace
    )
    # [N_CORES, NST, 128, PPP]
    pms = np.stack([res.results[i]["outp"] for i in range(N_CORES)])
    return pms, res


def _bbox_from_maxes(rowv, colv, thresh):
    """rowv [B,H], colv [B,W] float32 maxima -> bbox coords, matching _bbox."""
    row_any = rowv > thresh
    col_any = colv > thresh
    ys = np.arange(H, dtype=np.float32)
    xs = np.arange(W, dtype=np.float32)
    y_min = np.where(row_any, ys, np.float32(H)).min(axis=1)
    y_max = np.where(row_any, ys, np.float32(-1)).max(axis=1)
    x_min = np.where(col_any, xs, np.float32(W)).min(axis=1)
    x_max = np.where(col_any, xs, np.float32(-1)).max(axis=1)
    empty = ~row_any.any(axis=1)
    f32 = np.float32
    y_min = np.where(empty, f32(0.0), y_min).astype(np.float32)
    x_min = np.where(empty, f32(0.0), x_min).astype(np.float32)
    y_max = np.where(empty, f32(1.0), y_max).astype(np.float32)
    x_max = np.where(empty, f32(1.0), x_max).astype(np.float32)
    return y_min, x_min, y_max, x_max


def _penalty_from_pms(pms):
    """pms [N_CORES, NST, 128, PPP] -> scalar penalty (float32)."""
    # pms[c, st] covers sample 2c + (st % SPC); st//SPC==0 -> pred, ==1 -> true
    pm4 = pms.reshape(N_CORES, 2, SPC, NPART, 2, W)  # [c, tensor, s, p, r, w]
    pm4 = pm4.transpose(1, 0, 2, 3, 4, 5).reshape(2, B, NPART, 2, W)
    rowv = pm4.max(axis=4)            # [2, B, 128, 2] -> rows 2p+r
    rowv = rowv.reshape(2, B, H)
    colv = pm4.max(axis=(2, 3))       # [2, B, W]

    p = _bbox_from_maxes(rowv[0], colv[0], np.float32(THRESHOLD))
    t = _bbox_from_maxes(rowv[1], colv[1], np.float32(0.5))
    py_min, px_min, py_max, px_max = p
    ty_min, tx_min, ty_max, tx_max = t

    one = np.float32(1.0)
    pred_area = (py_max - py_min + one) * (px_max - px_min + one)
    true_area = (ty_max - ty_min + one) * (tx_max - tx_min + one)
    area_penalty = np.maximum(pred_area - true_area, np.float32(0.0)) / (
        true_area + one
    )
    two = np.float32(2.0)
    dy = (py_min + py_max) / two - (ty_min + ty_max) / two
    dx = (px_min + px_max) / two - (tx_min + tx_max) / two
    center_offset = np.sqrt(dy * dy + dx * dx).astype(np.float32) / np.float32(
        20.0
    )
    penalties = area_penalty + center_offset
    return np.float32(PENALTY_WEIGHT) * penalties.mean(dtype=np.float32)


def _run(prediction_probs, expected_onehot, trace=False):
    pred_np = np.ascontiguousarray(
        np.asarray(prediction_probs, dtype=np.float32)
    )
    true_np = np.ascontiguousarray(
        np.asarray(expected_onehot, dtype=np.float32)
    )
    assert pred_np.shape == (B, H, W, C), pred_np.shape
    assert true_np.shape == (B, H, W, C), true_np.shape
    pms, res = _run_device(pred_np, true_np, trace=trace)
    val = _penalty_from_pms(pms)
    return np.asarray(val, dtype=np.float32), res


def kernel(prediction_probs, expected_onehot):
    out, _ = _run(prediction_probs, expected_onehot, trace=False)
    return out


# revision 12
# speedup vs baseline: 1.2072x; 1.2072x over previous
"""Bounding-box discipline penalty kernel for Trainium2 (8 NeuronCores).

Reference computation:
    pred_mask = max_c(prediction_probs) > 0.3   [B, H, W]
    true_mask = max_c(expected_onehot)  > 0.5   [B, H, W]
    per-sample bboxes from the masks -> area/center penalties -> scalar mean.

Strategy (pure data parallel, B=16 over 8 cores => 2 samples/core):
  * Device: stream both tensors' shards through SBUF and compute the
    per-pixel channel max, laid out as pixmax[partition=128, 512] per
    (tensor, sample). That is the entire memory-bound part (reads 128 MiB
    per core at HBM line rate; the reduction overlaps the DMA stream).
    The last sample-tensor's chunks taper off in size and alternate
    between the Vector and GpSimd engines so the final reduction drains
    in parallel instead of serializing after the last DMA.
  * Host: fold the tiny [4, 128, 512] per-core results into per-sample
    row/col maxima (exact max operations, order-independent), then do the
    O(B) bbox + penalty math exactly as the reference does.

Self-contained: hardcodes shapes from the problem spec.
"""

import numpy as np

THRESHOLD = 0.3
PENALTY_WEIGHT = 0.05

B, H, W, C = 16, 256, 256, 128
N_CORES = 8
SPC = B // N_CORES            # samples per core = 2
NST = 2 * SPC                 # sample-tensor streams per core = 4
PIX = H * W                   # 65536 pixels per sample
NPART = 128
PPP = PIX // NPART            # 512 pixels per partition
EPP = PPP * C                 # 65536 f32 elems per partition per sample
NT = 4                        # full-size tiles per sample-tensor
F = EPP // NT                 # 16384 elems/partition per DMA (8 MiB tiles)
NB = 3                        # SBUF load-buffer ring depth

_cache = {}


def _chunk_schedule():
    """Chunk list [(st, elem offset, size, reducer)] with reducer 'v' | 'g'.

    Uniform 8 MiB chunks reduced on Vector. The last sample-tensor tapers
    off and alternates Vector/GpSimd so the tail reduction parallelizes.
    """
    per_st = []
    for st in range(NST):
        if st < NST - 1:
            per_st.append([(F, "v")] * NT)
        else:
            per_st.append(
                [
                    (F, "v"),
                    (F, "v"),
                    (F, "v"),
                    (F // 2, "v"),
                    (F // 4, "v"),
                    (F // 8, "v"),
                    (F // 16, "v"),
                    (F // 16, "v"),
                ]
            )
    loads = []
    for st, chunks in enumerate(per_st):
        off = 0
        for sz, red in chunks:
            loads.append((st, off, sz, red))
            off += sz
        assert off == EPP
    return per_st, loads


def _build_nc():
    from contextlib import ExitStack

    import concourse.bass as bass
    import concourse.mybir as mybir

    f32 = mybir.dt.float32
    nc = bass.Bass()
    pred = nc.dram_tensor("pred", [SPC, NPART, EPP], f32, kind="ExternalInput")
    tru = nc.dram_tensor("tru", [SPC, NPART, EPP], f32, kind="ExternalInput")
    # pixmax per sample-tensor: [st, partition, pixel-in-partition]
    outp = nc.dram_tensor("outp", [NST, NPART, PPP], f32, kind="ExternalOutput")

    srcs = [(pred, 0), (pred, 1), (tru, 0), (tru, 1)]
    per_st, loads = _chunk_schedule()

    # per-chunk owner sequence numbers (1-based count on the owner's sem)
    owner_seq = []
    counts = {"v": 0, "g": 0}
    for _st, _off, _sz, red in loads:
        counts[red] += 1
        owner_seq.append((red, counts[red]))
    # owner counts completed through the end of each st (for out flushes)
    thru = {"v": [], "g": []}
    cv = cg = 0
    for st, chunks in enumerate(per_st):
        for _sz, red in chunks:
            if red == "v":
                cv += 1
            else:
                cg += 1
        thru["v"].append(cv)
        thru["g"].append(cg)

    with ExitStack() as ctx:
        buf = [
            ctx.enter_context(nc.sbuf_tensor(f"buf{i}", [NPART, F], f32))
            for i in range(NB)
        ]
        pm = [
            ctx.enter_context(nc.sbuf_tensor(f"pm{i}", [NPART, PPP], f32))
            for i in range(NST)
        ]
        lsems = [
            ctx.enter_context(nc.semaphore(f"ls{i}")) for i in range(NB)
        ]
        vfree = ctx.enter_context(nc.semaphore("vfree"))
        gfree = ctx.enter_context(nc.semaphore("gfree"))
        outsem = ctx.enter_context(nc.semaphore("outsem"))
        block = ctx.enter_context(nc.Block())
        free_sems = {"v": vfree, "g": gfree}
        n_outs = 0

        @block.sync
        def _(sync):
            for k, (st, off, sz, _red) in enumerate(loads):
                src, s = srcs[st]
                if k >= NB:
                    # buffer k%NB reused from load k-NB: wait for its reduce
                    own, seq = owner_seq[k - NB]
                    sync.wait_ge(free_sems[own], seq)
                sync.dma_start(
                    out=buf[k % NB][:, :sz], in_=src[s, :, off : off + sz]
                ).then_inc(lsems[k % NB], 16)

        def reducer_prog(engine, which):
            for k, (st, off, sz, red) in enumerate(loads):
                if red != which:
                    continue
                engine.wait_ge(lsems[k % NB], 16 * (k // NB + 1))
                engine.reduce_max(
                    out=pm[st][:, off // C : (off + sz) // C],
                    in_=buf[k % NB][:, :sz].rearrange(
                        "p (a c) -> p a c", c=C
                    ),
                    axis=mybir.AxisListType.X,
                ).then_inc(free_sems[which], 1)

        @block.vector
        def _(vector):
            reducer_prog(vector, "v")

        @block.gpsimd
        def _(gpsimd):
            reducer_prog(gpsimd, "g")

        @block.scalar
        def _(scalar):
            nonlocal_outs = [0]

            def flush(st, px_lo, px_hi, need_v, need_g):
                if need_v:
                    scalar.wait_ge(vfree, need_v)
                if need_g:
                    scalar.wait_ge(gfree, need_g)
                scalar.dma_start(
                    out=outp[st, :, px_lo:px_hi],
                    in_=pm[st][:, px_lo:px_hi],
                ).then_inc(outsem, 16)
                nonlocal_outs[0] += 1

            for st in range(NST):
                chunks = per_st[st]
                if len(chunks) == NT:
                    flush(st, 0, PPP, thru["v"][st], thru["g"][st])
                else:
                    # tapered st: flush the first NT-1 full chunks' pixels
                    # early, then the remainder once everything is reduced
                    head_px = sum(sz for sz, _ in chunks[: NT - 1]) // C
                    base_v = thru["v"][st - 1] if st else 0
                    base_g = thru["g"][st - 1] if st else 0
                    nv = sum(1 for _sz, r in chunks[: NT - 1] if r == "v")
                    ng = sum(1 for _sz, r in chunks[: NT - 1] if r == "g")
                    flush(st, 0, head_px, base_v + nv, base_g + ng)
                    flush(st, head_px, PPP, thru["v"][st], thru["g"][st])
            scalar.wait_ge(outsem, 16 * nonlocal_outs[0])

    return nc


def _run_device(pred_np, true_np, trace=False):
    from concourse.bass_utils import run_bass_kernel_spmd

    if "nc" not in _cache:
        _cache["nc"] = _build_nc()
    nc = _cache["nc"]

    # [B, H, W, C] -> per-core shards [SPC, 128, EPP]
    pred_sh = pred_np.reshape(N_CORES, SPC, NPART, EPP)
    true_sh = true_np.reshape(N_CORES, SPC, NPART, EPP)
    in_maps = [
        {"pred": pred_sh[i], "tru": true_sh[i]} for i in range(N_CORES)
    ]
    res = run_bass_kernel_spmd(
        nc, in_maps, core_ids=list(range(N_CORES)), trace=trace
    )
    # [N_CORES, NST, 128, PPP]
    pms = np.stack([res.results[i]["outp"] for i in range(N_CORES)])
    return pms, res


def _bbox_from_maxes(rowv, colv, thresh):
    """rowv [B,H], colv [B,W] float32 maxima -> bbox coords, matching _bbox."""
    row_any = rowv > thresh
    col_any = colv > thresh
    ys = np.arange(H, dtype=np.float32)
    xs = np.arange(W, dtype=np.float32)
    y_min = np.where(row_any, ys, np.float32(H)).min(axis=1)
    y_max = np.where(row_any, ys, np.float32(-1)).max(axis=1)
    x_min = np.where(col_any, xs, np.float32(W)).min(axis=1)
    x_max = np.where(col_any, xs, np.float32(-1)).max(axis=1)
    empty = ~row_any.any(axis=1)
    f32 = np.float32
    y_min = np.where(empty, f32(0.0), y_min).astype(np.float32)
    x_min = np.where(empty, f32(0.0), x_min).astype(np.float32)
    y_max = np.where(empty, f32(1.0), y_max).astype(np.float32)
    x_max = np.where(empty, f32(1.0), x_max).astype(np.float32)
    return y_min, x_min, y_max, x_max


def _penalty_from_pms(pms):
    """pms [N_CORES, NST, 128, PPP] -> scalar penalty (float32)."""
    # pms[c, st] covers sample 2c + (st % SPC); st//SPC==0 -> pred, ==1 -> true
    pm4 = pms.reshape(N_CORES, 2, SPC, NPART, 2, W)  # [c, tensor, s, p, r, w]
    pm4 = pm4.transpose(1, 0, 2, 3, 4, 5).reshape(2, B, NPART, 2, W)
    rowv = pm4.max(axis=4)            # [2, B, 128, 2] -> rows 2p+r
    rowv = rowv.reshape(2, B, H)
    colv = pm4.max(axis=(2, 3))       # [2, B, W]

    p = _bbox_from_maxes(rowv[0], colv[0], np.float32(THRESHOLD))
    t = _bbox_from_maxes(rowv[1], colv[1], np.float32(0.5))
    py_min, px_min, py_max, px_max = p
    ty_min, tx_min, ty_max, tx_max = t

    one = np.float32(1.0)
    pred_area = (py_max - py_min + one) * (px_max - px_min + one)
    true_area = (ty_max - ty_min + one) * (tx_max - tx_min + one)
    area_penalty = np.maximum(pred_area - true_area, np.float32(0.0)) / (
        true_area + one
    )
    two = np.float32(2.0)
    dy = (py_min + py_max) / two - (ty_min + ty_max) / two
    dx = (px_min + px_max) / two - (tx_min + tx_max) / two
    center_offset = np.sqrt(dy * dy + dx * dx).astype(np.float32) / np.float32(
        20.0
    )
    penalties = area_penalty + center_offset
    return np.float32(PENALTY_WEIGHT) * penalties.mean(dtype=np.float32)


def _run(prediction_probs, expected_onehot, trace=False):
    pred_np = np.ascontiguousarray(
        np.asarray(prediction_probs, dtype=np.float32)
    )
    true_np = np.ascontiguousarray(
        np.asarray(expected_onehot, dtype=np.float32)
    )
    assert pred_np.shape == (B, H, W, C), pred_np.shape
    assert true_np.shape == (B, H, W, C), true_np.shape
    pms, res = _run_device(pred_np, true_np, trace=trace)
    val = _penalty_from_pms(pms)
    return np.asarray(val, dtype=np.float32), res


def kernel(prediction_probs, expected_onehot):
    out, _ = _run(prediction_probs, expected_onehot, trace=False)
    return out
